# revision 1
# baseline (speedup 1.0000x reference)
"""MultiHeadAttention Trainium2 kernel.

Strategy: data-parallel, one batch element per NeuronCore (B=8 == n_cores).
Per core (batch b):
  - Host pre-transposes x_Q/x_K/x_V[b] -> xT [D, S] (bf16) and weights -> W^T (bf16),
    so every matmul has its contraction dim on partitions.
  - Projections on PE: Q^T/K^T in [hd, s] layout (head pairs stacked on 128
    partitions), V in natural [s, hd] layout grouped per head with an extra
    ones column (gives the softmax denominator for free during P@V).
  - scores^T [k, q] per (head, k-tile): QK^T with K=64 contraction; exp on
    ScalarE reading PSUM directly, applying scale=1/sqrt(64) and the key
    padding mask as a per-partition bias (-1e9 -> exp underflows to exact 0).
  - P@V with lhsT = P^T tile (M=128 queries) -> ctx in natural [q, hd] layout
    with denominator column; normalization is a per-partition reciprocal +
    broadcast multiply on VectorE (fused with the PSUM->SBUF copy).
  - ctx transposed via DMA xbar (bf16) for the output projection; final
    out = ctx @ Wo^T + bo in fp32.
Masked key tiles are computed but contribute exactly 0 (uniform SPMD program,
robust to any src_batch_lens values).
"""

import numpy as np
import ml_dtypes

import concourse.bass as bass  # noqa: F401
import concourse.tile as tile
from concourse import bacc, mybir
from concourse._compat import get_trn_type
from concourse.bass_utils import run_bass_kernel_spmd

B, S, D = 8, 2048, 512
H, DH = 8, 64
P = 128
NDT = D // P      # 4 tiles over the model/hd dim
NST = S // P      # 16 tiles over sequence (queries)
NKT = S // P      # 16 tiles over keys
F32 = mybir.dt.float32
BF16 = mybir.dt.bfloat16
NEG = -1.0e9

# stash for test.py introspection
last_results = None


DEBUG = False


def _build_program():
    nc = bacc.Bacc(get_trn_type() or "TRN2", target_bir_lowering=False)

    xqT_d = nc.dram_tensor("xqT", (P, NDT, S), BF16, kind="ExternalInput")
    xkT_d = nc.dram_tensor("xkT", (P, NDT, S), BF16, kind="ExternalInput")
    xvT_d = nc.dram_tensor("xvT", (P, NDT, S), BF16, kind="ExternalInput")
    wqT_d = nc.dram_tensor("wqT", (P, NDT, D), BF16, kind="ExternalInput")
    wkT_d = nc.dram_tensor("wkT", (P, NDT, D), BF16, kind="ExternalInput")
    wvT_d = nc.dram_tensor("wvT", (P, NDT, D), BF16, kind="ExternalInput")
    woT_d = nc.dram_tensor("woT", (P, NDT, D), BF16, kind="ExternalInput")
    bqT_d = nc.dram_tensor("bqT", (P, NDT), F32, kind="ExternalInput")
    bkT_d = nc.dram_tensor("bkT", (P, NDT), F32, kind="ExternalInput")
    bv_d = nc.dram_tensor("bvb", (P, D), F32, kind="ExternalInput")
    bo_d = nc.dram_tensor("bob", (P, D), F32, kind="ExternalInput")
    mask_d = nc.dram_tensor("mask", (P, NKT), F32, kind="ExternalInput")
    out_d = nc.dram_tensor("out", (P, NST, D), F32, kind="ExternalOutput")
    if DEBUG:
        qdbg_d = nc.dram_tensor("qdbg", (P, NDT, S), BF16, kind="ExternalOutput")
        kdbg_d = nc.dram_tensor("kdbg", (P, NDT, S), BF16, kind="ExternalOutput")
        vdbg_d = nc.dram_tensor("vdbg", (P, NST, H, DH + 1), BF16, kind="ExternalOutput")
        ctdbg_d = nc.dram_tensor("ctdbg", (P, NDT, S), BF16, kind="ExternalOutput")
        pdbg_d = nc.dram_tensor("pdbg", (P, NKT, 1024), BF16, kind="ExternalOutput")

    Exp = mybir.ActivationFunctionType.Exp
    MUL = mybir.AluOpType.mult
    ADD = mybir.AluOpType.add

    with tile.TileContext(nc) as tc:
        with tc.tile_pool(name="persist", bufs=1) as pp:
            wo_sb = pp.tile([P, NDT, D], BF16, tag="wo")
            nc.sync.dma_start(wo_sb[:], woT_d[:])
            mask_sb = pp.tile([P, NKT], F32, tag="mask")
            nc.sync.dma_start(mask_sb[:], mask_d[:])
            bqT_sb = pp.tile([P, NDT], F32, tag="bqT")
            nc.sync.dma_start(bqT_sb[:], bqT_d[:])
            bkT_sb = pp.tile([P, NDT], F32, tag="bkT")
            nc.sync.dma_start(bkT_sb[:], bkT_d[:])
            bv_sb = pp.tile([P, D], F32, tag="bv")
            nc.sync.dma_start(bv_sb[:], bv_d[:])
            bo_sb = pp.tile([P, D], F32, tag="bo")
            nc.sync.dma_start(bo_sb[:], bo_d[:])

            qT_sb = pp.tile([P, NDT, S], BF16, tag="qT")
            kT_sb = pp.tile([P, NDT, S], BF16, tag="kT")
            v_sb = pp.tile([P, NST, H, DH + 1], BF16, tag="v")
            ctxT_sb = pp.tile([P, NDT, S], BF16, tag="ctxT")

            # ones column for the denominator trick
            nc.vector.memset(v_sb[:, :, :, DH : DH + 1], 1.0)
            # e64: selects the denominator row (partition 64) in the
            # partition-broadcast matmul
            e64_sb = pp.tile([DH + 1, DH], BF16, tag="e64")
            nc.vector.memset(e64_sb[:], 0.0)
            nc.vector.memset(e64_sb[DH : DH + 1, :], 1.0)

            # ---- Shared PSUM pools (exactly 8 banks total) ----
            _scp_cm = tc.tile_pool(name="scps", bufs=2, space="PSUM")
            _cxp_cm = tc.tile_pool(name="cxps", bufs=2, space="PSUM")
            scp = _scp_cm.__enter__()
            cxp = _cxp_cm.__enter__()

            # ---- Phase 1: input loads + first projection tiles ----
            _xp_cm = tc.tile_pool(name="xin", bufs=1)
            xp = _xp_cm.__enter__()
            xq_sb = xp.tile([P, NDT, S], BF16, tag="xq")
            nc.sync.dma_start(xq_sb[:, :, 0:1024], xqT_d[:, :, 0:1024])
            nc.sync.dma_start(xq_sb[:, :, 1024:2048], xqT_d[:, :, 1024:2048])
            xk_sb = xp.tile([P, NDT, S], BF16, tag="xk")
            nc.sync.dma_start(xk_sb[:, :, 0:1024], xkT_d[:, :, 0:1024])
            nc.sync.dma_start(xk_sb[:, :, 1024:2048], xkT_d[:, :, 1024:2048])
            xv_sb = xp.tile([P, NDT, S], BF16, tag="xv")
            nc.sync.dma_start(xv_sb[:, :, 0:1024], xvT_d[:, :, 0:1024])
            nc.sync.dma_start(xv_sb[:, :, 1024:2048], xvT_d[:, :, 1024:2048])
            wq_sb = xp.tile([P, NDT, D], BF16, tag="wq")
            nc.sync.dma_start(wq_sb[:], wqT_d[:])
            wk_sb = xp.tile([P, NDT, D], BF16, tag="wk")
            nc.sync.dma_start(wk_sb[:], wkT_d[:])
            wv_sb = xp.tile([P, NDT, D], BF16, tag="wv")
            nc.sync.dma_start(wv_sb[:], wvT_d[:])

            def qk_proj_group(w_sb, x_sb, o_sb, b_sb, mt, half, pool, tag):
                # one [128, 1024] chunk of Q^T or K^T: out^T = (W^T).T @ x^T
                ps = pool.tile([P, 2, 512], F32, tag=tag, name="pj")
                for qc in range(2):
                    for kt in range(NDT):
                        nc.tensor.matmul(
                            ps[:, qc, :],
                            lhsT=w_sb[:, kt, mt * P : (mt + 1) * P],
                            rhs=x_sb[:, kt, half * 1024 + qc * 512 : half * 1024 + (qc + 1) * 512],
                            start=(kt == 0),
                            stop=(kt == NDT - 1),
                        )
                nc.vector.tensor_scalar_add(
                    o_sb[:, mt, half * 1024 : (half + 1) * 1024],
                    ps[:].rearrange("p a b -> p (a b)"),
                    b_sb[:, mt : mt + 1],
                )

            def v_proj_group(st, pool, tag):
                # V natural [s, hd] head-grouped with bias
                psv = pool.tile([P, 2, 512], F32, tag=tag, name="pv")
                for kt in range(NDT):
                    nc.tensor.matmul(
                        psv[:, 0, :],
                        lhsT=xv_sb[:, kt, st * P : (st + 1) * P],
                        rhs=wv_sb[:, kt, :],
                        start=(kt == 0),
                        stop=(kt == NDT - 1),
                    )
                nc.vector.tensor_tensor(
                    out=v_sb[:, st, :, 0:DH],
                    in0=psv[:, 0, :].rearrange("p (h d) -> p h d", h=H),
                    in1=bv_sb[:].rearrange("p (h d) -> p h d", h=H),
                    op=ADD,
                )

            # head-pair 0 (Mtile 0) of Q^T and K^T up front; the rest is
            # interleaved into the attention head loops below
            for half in range(2):
                qk_proj_group(wq_sb, xq_sb, qT_sb, bqT_sb, 0, half, scp, "sc")
                qk_proj_group(wk_sb, xk_sb, kT_sb, bkT_sb, 0, half, scp, "sc")
            # remaining projection groups, consumed inside the attention loops
            projq = []
            for mt in range(1, NDT):
                for half in range(2):
                    projq.append((wq_sb, xq_sb, qT_sb, bqT_sb, mt, half))
                    projq.append((wk_sb, xk_sb, kT_sb, bkT_sb, mt, half))

            # ---- Phase 2+3: attention, out-projection per q-half ----
            with tc.tile_pool(name="ptp", bufs=8) as ptp, \
                 tc.tile_pool(name="cup", bufs=4) as cup, \
                 tc.tile_pool(name="rrp", bufs=3) as rrp, \
                 tc.tile_pool(name="obp", bufs=4) as obp:
                def oproj_group(st, pool, tag):
                    pso = pool.tile([P, 2, 512], F32, tag=tag, name="pso")
                    for kt in range(NDT):
                        nc.tensor.matmul(
                            pso[:, 0, :],
                            lhsT=ctxT_sb[:, kt, st * P : (st + 1) * P],
                            rhs=wo_sb[:, kt, :],
                            start=(kt == 0),
                            stop=(kt == NDT - 1),
                        )
                    ot = obp.tile([P, D], F32, tag="ot")
                    nc.vector.tensor_tensor(out=ot[:], in0=pso[:, 0, :], in1=bo_sb[:], op=ADD)
                    nc.sync.dma_start(out_d[:, st, :], ot[:])

                oprojq = []
                pending_flush = [None]

                def run_pending():
                    if pending_flush[0] is not None:
                        pending_flush[0]()
                        pending_flush[0] = None

                for qh in range(2):
                    q0 = qh * 1024
                    for h in range(H):
                        pbase = (h % 2) * 64
                        hm = h // 2
                        # ctx^T accumulator rows 0..63 = head dims, row 64 =
                        # softmax denominator (ones column of V')
                        cxt = cxp.tile([P, 2, 512], F32, tag="cx")

                        def pv_step(t, pt):
                            for qc in range(2):
                                nc.tensor.matmul(
                                    cxt[0 : DH + 1, qc, :],
                                    lhsT=v_sb[:, t, h, :],
                                    rhs=pt[:, qc * 512 : (qc + 1) * 512],
                                    start=(t == 0),
                                    stop=(t == NKT - 1),
                                )

                        # software pipeline: PV for tile t-1 is emitted after
                        # QK/exp for tile t, so the PE never stalls behind ACT
                        prev = None
                        for t in range(NKT):
                            if qh == 0 and h == 0:
                                # V tile t is produced just in time for PV
                                v_proj_group(t, cxp, "cx")
                            pop_proj = (h == 1 and t % 4 == 2) or (
                                2 <= h <= 5 and t % 8 == 2
                            )
                            if qh == 0 and pop_proj and projq:
                                g = projq.pop(0)
                                qk_proj_group(*g, cxp, "cx")
                            if qh == 1 and t % 4 == 3 and oprojq:
                                oproj_group(oprojq.pop(0), cxp, "cx")
                            if t == 1:
                                # previous head's flush, deferred so its PE ops
                                # don't stall the stream at the head boundary
                                run_pending()
                            sc = scp.tile([P, 2, 512], F32, tag="sc")
                            for qc in range(2):
                                nc.tensor.matmul(
                                    sc[:, qc, :],
                                    lhsT=kT_sb[pbase : pbase + 64, hm, t * P : (t + 1) * P],
                                    rhs=qT_sb[pbase : pbase + 64, hm, q0 + qc * 512 : q0 + (qc + 1) * 512],
                                    start=True,
                                    stop=True,
                                )
                            pt = ptp.tile([P, 1024], BF16, tag="pt")
                            nc.scalar.activation(
                                pt[:],
                                sc[:].rearrange("p a b -> p (a b)"),
                                Exp,
                                bias=mask_sb[:, t : t + 1],
                                scale=0.125,
                            )
                            if DEBUG and h == 0 and qh == 0:
                                nc.sync.dma_start(pdbg_d[:, t, :], pt[:])
                            if prev is not None:
                                pv_step(*prev)
                            prev = (t, pt)
                        pv_step(*prev)
                        # flush: copy to SBUF, broadcast denom via PE, recip,
                        # normalize into ctx^T. The SBUF copy is emitted now
                        # (frees the cx slot); the PE/recip part is deferred
                        # into the next head's loop.
                        cu = cup.tile([DH + 1, 2, 512], BF16, tag="cu")
                        nc.vector.tensor_copy(cu[:], cxt[0 : DH + 1])

                        def flush(cu=cu, pbase=pbase, hm=hm, q0=q0):
                            rb = cxp.tile([P, 2, 512], F32, tag="cx", name="rb")
                            for qc in range(2):
                                nc.tensor.matmul(
                                    rb[0:DH, qc, :],
                                    lhsT=e64_sb[:],
                                    rhs=cu[:, qc, :],
                                    start=True,
                                    stop=True,
                                )
                            rc = rrp.tile([DH, 2, 512], F32, tag="rc")
                            nc.vector.reciprocal(rc[:], rb[0:DH])
                            nc.vector.tensor_tensor(
                                out=ctxT_sb[pbase : pbase + 64, hm, q0 : q0 + 1024],
                                in0=cu[0:DH].rearrange("p a b -> p (a b)"),
                                in1=rc[:].rearrange("p a b -> p (a b)"),
                                op=MUL,
                            )

                        run_pending()
                        pending_flush[0] = flush

                    # output projection: qh0's s-tiles are queued and
                    # interleaved into qh1's head loops; qh1's form the tail
                    run_pending()
                    if qh == 0:
                        oprojq.extend(range(0, 8))
                    else:
                        for st in oprojq:
                            oproj_group(st, scp, "sc")
                        oprojq = []
                        for st in range(8, 16):
                            oproj_group(st, scp, "sc")

            _xp_cm.__exit__(None, None, None)
            _cxp_cm.__exit__(None, None, None)
            _scp_cm.__exit__(None, None, None)

            if DEBUG:
                nc.sync.dma_start(qdbg_d[:], qT_sb[:])
                nc.sync.dma_start(kdbg_d[:], kT_sb[:])
                nc.sync.dma_start(vdbg_d[:], v_sb[:])
                nc.sync.dma_start(ctdbg_d[:], ctxT_sb[:])

    nc.compile()
    return nc


_program_cache = None


def _get_program():
    global _program_cache
    if _program_cache is None:
        _program_cache = _build_program()
    return _program_cache


def _to_bf16_T_tiled(x):
    # [S, D] fp32 -> x^T [D, S] -> [P, NDT, S] bf16 with d = dt*128 + p
    xt = np.ascontiguousarray(x.T.astype(ml_dtypes.bfloat16))
    return np.ascontiguousarray(xt.reshape(NDT, P, S).transpose(1, 0, 2))


def _w_T_tiled(w):
    # torch Linear weight [out, in] -> W^T [in, out] -> [P, NDT, out] bf16
    wt = np.ascontiguousarray(w.T.astype(ml_dtypes.bfloat16))
    return np.ascontiguousarray(wt.reshape(NDT, P, w.shape[0]).transpose(1, 0, 2))


def kernel(**inputs):
    global last_results
    x_Q = np.asarray(inputs["x_Q"], dtype=np.float32)
    x_K = np.asarray(inputs["x_K"], dtype=np.float32)
    x_V = np.asarray(inputs["x_V"], dtype=np.float32)
    Wq = np.asarray(inputs["Wq"], dtype=np.float32)
    Wk = np.asarray(inputs["Wk"], dtype=np.float32)
    Wv = np.asarray(inputs["Wv"], dtype=np.float32)
    Wo = np.asarray(inputs["Wo"], dtype=np.float32)
    bq = np.asarray(inputs["bq"], dtype=np.float32)
    bk = np.asarray(inputs["bk"], dtype=np.float32)
    bv = np.asarray(inputs["bv"], dtype=np.float32)
    bo = np.asarray(inputs["bo"], dtype=np.float32)
    lens = np.asarray(inputs["src_batch_lens"]).astype(np.int64)

    nc = _get_program()

    wqT = _w_T_tiled(Wq)
    wkT = _w_T_tiled(Wk)
    wvT = _w_T_tiled(Wv)
    woT = _w_T_tiled(Wo)
    bqT = np.ascontiguousarray(bq.reshape(NDT, P).T).astype(np.float32)
    bkT = np.ascontiguousarray(bk.reshape(NDT, P).T).astype(np.float32)
    bvb = np.ascontiguousarray(np.broadcast_to(bv, (P, D))).astype(np.float32)
    bob = np.ascontiguousarray(np.broadcast_to(bo, (P, D))).astype(np.float32)

    in_maps = []
    for b in range(B):
        kpos = (np.arange(NKT * P).reshape(NKT, P).T).astype(np.int64)  # [P, NKT]
        mask = np.where(kpos < lens[b], 0.0, NEG).astype(np.float32)
        in_maps.append(
            {
                "xqT": _to_bf16_T_tiled(x_Q[b]),
                "xkT": _to_bf16_T_tiled(x_K[b]),
                "xvT": _to_bf16_T_tiled(x_V[b]),
                "wqT": wqT,
                "wkT": wkT,
                "wvT": wvT,
                "woT": woT,
                "bqT": bqT,
                "bkT": bkT,
                "bvb": bvb,
                "bob": bob,
                "mask": np.ascontiguousarray(mask),
            }
        )

    res = run_bass_kernel_spmd(nc, in_maps, core_ids=list(range(B)))
    last_results = res

    out = np.empty((B, S, D), dtype=np.float32)
    for b in range(B):
        o = res.results[b]["out"]  # [P, NST, D]
        out[b] = o.transpose(1, 0, 2).reshape(S, D)
    return out



# revision 36
# speedup vs baseline: 1.4477x; 1.4477x over previous
"""MultiHeadAttention Trainium2 kernel (lens-balanced, schedule-specialized).

Sharding: the 8 batches are sorted by key-tile count T_b = ceil(len_b/128)
and paired (rank0,rank1), (rank2,rank3), ... giving 4 "slots" with
compile-time tile counts T_s = max of the pair.  Every core runs the same
program: one 512-query quarter from one batch of each pair (cores 0-3 take
the first batch of each pair, cores 4-7 the second).  Per-core work is
Sigma_s T_s tiles instead of the data-parallel max_b T_b * 4.

Per (slot, head) on the device:
  - scores^T [k,q] per key tile via QK^T (contraction dh=64), exp on ACT
    reading PSUM with scale=1/8; fully-valid key tiles are exp'd in pairs
    (one [128,2,512] instruction), boundary/padding tiles singly with the
    key mask as a per-partition bias (-1e9 -> exact 0).
  - PV flipped: ctx[128q, 65] += P_tile^T (stationary) @ V' (moving, 65
    cols incl. a ones column for the softmax denominator).  All 4 q-tiles
    of the slot pack into ONE PSUM bank ([128,4,128] f32).
  - normalize on DVE (reciprocal of col 64 + per-qtile scalar multiply),
    ctx^T for the output projection via DMA-xbar transposes of [128,128]
    head-pair tiles.
  - output projection accumulates 4 hd-tiles into PSUM, DMA'd straight
    from PSUM to DRAM (biases are all zero in this model; a runtime check
    falls back to bias-add paths if not).

The program is compiled per (lens, bias-zero) signature; masks and slot
assignment are computed on the host from the runtime src_batch_lens.
"""

import numpy as np
import ml_dtypes

import concourse.bass as bass  # noqa: F401
import concourse.tile as tile
from concourse import bacc, mybir
from concourse._compat import get_trn_type
from concourse.bass_utils import run_bass_kernel_spmd

B, S, D = 8, 2048, 512
H, DH = 8, 64
P = 128
NDT = D // P      # 4 tiles over the model/hd dim
NQT = S // P      # 16 query tiles per batch
QW = 512          # query width per slot (quarter batch)
NSLOT = 4
F32 = mybir.dt.float32
BF16 = mybir.dt.bfloat16
NEG = -1.0e9

# stash for test.py introspection
last_results = None


def _schedule_from_lens(lens):
    """lens -> (order, tsched, tfull) with order the batch ids sorted by
    descending tile count, tsched[p] = tiles of pair p's longer batch,
    tfull[p] = number of key tiles valid for BOTH batches of the pair."""
    lens = np.asarray(lens, dtype=np.int64)
    tiles = np.maximum((lens + P - 1) // P, 1)
    order = np.argsort(-tiles, kind="stable")
    tsched, tfull = [], []
    for p in range(NSLOT):
        a, b = order[2 * p], order[2 * p + 1]
        tsched.append(int(tiles[a]))
        tfull.append(int(min(lens[a], lens[b]) // P))
    return [int(x) for x in order], tsched, tfull


def _build_program(tsched, tfull, zero_bias):
    nc = bacc.Bacc(get_trn_type() or "TRN2", target_bir_lowering=False)
    ktot = sum(tsched)
    tA = tsched[0]
    tB = max(tsched[1:]) if NSLOT > 1 else tsched[0]

    xqT_d = nc.dram_tensor("xqT", (P, NDT, S), BF16, kind="ExternalInput")
    xkT_d = nc.dram_tensor("xkT", (P, NDT, ktot * P), BF16, kind="ExternalInput")
    xvT_d = nc.dram_tensor("xvT", (P, NDT, ktot * P), BF16, kind="ExternalInput")
    wqT_d = nc.dram_tensor("wqT", (P, NDT, D), BF16, kind="ExternalInput")
    wkT_d = nc.dram_tensor("wkT", (P, NDT, D), BF16, kind="ExternalInput")
    wvT_d = nc.dram_tensor("wvT", (P, NDT, D), BF16, kind="ExternalInput")
    woT_d = nc.dram_tensor("woT", (P, NDT, D), BF16, kind="ExternalInput")
    mask_d = nc.dram_tensor("mask", (P, ktot), F32, kind="ExternalInput")
    if not zero_bias:
        bqT_d = nc.dram_tensor("bqT", (P, NDT), F32, kind="ExternalInput")
        bkT_d = nc.dram_tensor("bkT", (P, NDT), F32, kind="ExternalInput")
        bv_d = nc.dram_tensor("bvb", (P, D), F32, kind="ExternalInput")
        bo_d = nc.dram_tensor("bob", (P, D), F32, kind="ExternalInput")
    out_d = nc.dram_tensor("out", (P, NQT, D), F32, kind="ExternalOutput")

    Exp = mybir.ActivationFunctionType.Exp
    MUL = mybir.AluOpType.mult
    ADD = mybir.AluOpType.add

    slot_off = np.cumsum([0] + tsched)  # key-tile offset of each slot

    with tile.TileContext(nc) as tc:
        with tc.tile_pool(name="persist", bufs=1) as pp:
            wq_sb = pp.tile([P, NDT, D], BF16, tag="wq")
            nc.sync.dma_start(wq_sb[:], wqT_d[:])
            wk_sb = pp.tile([P, NDT, D], BF16, tag="wk")
            wv_sb = pp.tile([P, NDT, D], BF16, tag="wv")
            wo_sb = pp.tile([P, NDT, D], BF16, tag="wo")
            mask_sb = pp.tile([P, ktot], F32, tag="mask")
            if not zero_bias:
                bqT_sb = pp.tile([P, NDT], F32, tag="bqT")
                nc.sync.dma_start(bqT_sb[:], bqT_d[:])
                bkT_sb = pp.tile([P, NDT], F32, tag="bkT")
                nc.sync.dma_start(bkT_sb[:], bkT_d[:])
                bv_sb = pp.tile([P, D], F32, tag="bv")
                nc.sync.dma_start(bv_sb[:], bv_d[:])
                bo_sb = pp.tile([P, D], F32, tag="bo")
                nc.sync.dma_start(bo_sb[:], bo_d[:])

            qT_sb = pp.tile([P, NDT, S], BF16, tag="qT")

            # ---- PSUM pools: 4 (scores) + 2 (ctx) + 2 (flex) = 8 banks ----
            _cms = []

            def _pool(**kw):
                cm = tc.tile_pool(**kw)
                _cms.append(cm)
                return cm.__enter__()

            scp = _pool(name="scps", bufs=2, space="PSUM")   # 2x2 banks: scores
            cxp = _pool(name="cxps", bufs=2, space="PSUM")   # 2x1 bank: ctx
            flx = _pool(name="flxps", bufs=2, space="PSUM")  # 2x1 bank: proj

            # ---- input / working pools ----
            xp = _pool(name="xin", bufs=4)
            kvA = _pool(name="kvA", bufs=1)
            kvB = _pool(name="kvB", bufs=1)
            ptp = _pool(name="ptp", bufs=2)
            rcp = _pool(name="rcp", bufs=4)
            csp = _pool(name="csp", bufs=4)
            otp = _pool(name="otp", bufs=4)
            ctp = _pool(name="ctp", bufs=2)

            def kv_bufs(s):
                """(xk, xv, kT, v) SBUF tiles for slot s (A/B rotation)."""
                pool, t = (kvA, tA) if s % 2 == 0 else (kvB, tB)
                xk = pool.tile([P, NDT, t * P], BF16, tag="xk")
                xv = pool.tile([P, NDT, t * P], BF16, tag="xv")
                kT = pool.tile([P, NDT, t * P], BF16, tag="kT")
                v = pool.tile([P, t, H, DH + 1], BF16, tag="v")
                return xk, xv, kT, v

            slot_bufs = {}

            def load_slot(s):
                xk, xv, kT, v = kv_bufs(s)
                o0, o1 = slot_off[s] * P, slot_off[s + 1] * P
                # chunked so the first projection can start before the
                # whole slice has landed
                for c0 in range(0, o1 - o0, 1024):
                    c1 = min(c0 + 1024, o1 - o0)
                    nc.sync.dma_start(xk[:, :, c0:c1], xkT_d[:, :, o0 + c0 : o0 + c1])
                    nc.sync.dma_start(xv[:, :, c0:c1], xvT_d[:, :, o0 + c0 : o0 + c1])
                slot_bufs[s] = (xk, xv, kT, v)

            # Deferred projection work. Each item carries:
            #   ready: earliest (slot, head) position it may run at (buffer
            #          anti-dependency safety -- running earlier would emit a
            #          PE instruction that waits on later-queued PE work)
            #   due:   position by which it MUST have been emitted (data
            #          dependency of the attention stream)
            #   est:   PE-ns estimate for budget-based pumping
            work_q = []

            def enq(ready, due, est, fn, args):
                work_q.append({"ready": ready, "due": due, "est": est,
                               "fn": fn, "args": args})

            def pump_due(pos):
                i = 0
                while i < len(work_q):
                    if work_q[i]["due"] <= pos:
                        it = work_q.pop(i)
                        it["fn"](*it["args"])
                    else:
                        i += 1

            def pump_budget(pos, ns):
                while ns > 0:
                    for i, it in enumerate(work_q):
                        if it["ready"] <= pos:
                            work_q.pop(i)
                            it["fn"](*it["args"])
                            ns -= it["est"]
                            break
                    else:
                        return

            def k_chunk(s, mt, c0, c1):
                _, _, kT, _ = slot_bufs[s]
                xk = slot_bufs[s][0]
                ps = flx.tile([P, QW], F32, tag="fx", name="kp")
                for kt in range(NDT):
                    nc.tensor.matmul(
                        ps[:, : c1 - c0],
                        lhsT=wk_sb[:, kt, mt * P : (mt + 1) * P],
                        rhs=xk[:, kt, c0:c1],
                        start=(kt == 0),
                        stop=(kt == NDT - 1),
                    )
                if zero_bias:
                    nc.vector.tensor_copy(kT[:, mt, c0:c1], ps[:, : c1 - c0])
                else:
                    nc.vector.tensor_scalar_add(
                        kT[:, mt, c0:c1], ps[:, : c1 - c0], bkT_sb[:, mt : mt + 1]
                    )

            def v_tile(s, kt):
                xv, v = slot_bufs[s][1], slot_bufs[s][3]
                ps = flx.tile([P, QW], F32, tag="fx", name="vp")
                for dt_ in range(NDT):
                    nc.tensor.matmul(
                        ps[:],
                        lhsT=xv[:, dt_, kt * P : (kt + 1) * P],
                        rhs=wv_sb[:, dt_, :],
                        start=(dt_ == 0),
                        stop=(dt_ == NDT - 1),
                    )
                if zero_bias:
                    nc.vector.tensor_copy(
                        v[:, kt, :, 0:DH],
                        ps[:].rearrange("p (h d) -> p h d", h=H),
                    )
                else:
                    nc.vector.tensor_tensor(
                        out=v[:, kt, :, 0:DH],
                        in0=ps[:].rearrange("p (h d) -> p h d", h=H),
                        in1=bv_sb[:].rearrange("p (h d) -> p h d", h=H),
                        op=ADD,
                    )

            def ones_col(s):
                v, t = slot_bufs[s][3], tsched[s]
                nc.vector.memset(v[:, :t, :, DH : DH + 1], 1.0)

            def q_chunk(qc, mt):
                xq = xq_tiles[qc]  # loaded by load_xq(qc)
                ps = flx.tile([P, QW], F32, tag="fx", name="qp")
                for kt in range(NDT):
                    nc.tensor.matmul(
                        ps[:],
                        lhsT=wq_sb[:, kt, mt * P : (mt + 1) * P],
                        rhs=xq[:, kt, :],
                        start=(kt == 0),
                        stop=(kt == NDT - 1),
                    )
                if zero_bias:
                    nc.vector.tensor_copy(qT_sb[:, mt, qc * QW : (qc + 1) * QW], ps[:])
                else:
                    nc.vector.tensor_scalar_add(
                        qT_sb[:, mt, qc * QW : (qc + 1) * QW],
                        ps[:],
                        bqT_sb[:, mt : mt + 1],
                    )

            def o_tile(s, j):
                ctxT = slot_ctxT[s]
                ps = flx.tile([P, QW], F32, tag="fx", name="op")
                for mt in range(NDT):
                    nc.tensor.matmul(
                        ps[:],
                        lhsT=ctxT[:, mt, j * P : (j + 1) * P],
                        rhs=wo_sb[:, mt, :],
                        start=(mt == 0),
                        stop=(mt == NDT - 1),
                    )
                ot = otp.tile([P, D], F32, tag="ot")
                if zero_bias:
                    nc.vector.tensor_copy(ot[:], ps[:])
                else:
                    nc.vector.tensor_tensor(out=ot[:], in0=ps[:], in1=bo_sb[:], op=ADD)
                nc.sync.dma_start(out_d[:, s * 4 + j, :], ot[:])

            def enq_kv_proj(s, ready):
                t = tsched[s]
                for kt in range(t):
                    enq(ready, (s, 1), 853, v_tile, (s, kt))
                enq(ready, (s, 1), 100, ones_col, (s,))
                for mt in range(1 if s == 0 else 0, NDT):
                    for c0 in range(0, t * P, QW):
                        c1 = min(c0 + QW, t * P)
                        enq(ready, (s, max(0, 2 * mt - 1)), 853, k_chunk, (s, mt, c0, c1))

            # ---- phase 0: minimal prologue, everything else deferred ----
            # DMA order: what head 0 of slot 0 needs comes first.
            xq_tiles = {}

            def load_xq(qc):
                xq = xp.tile([P, NDT, QW], BF16, tag="xq", name="xq")
                xq_tiles[qc] = xq
                nc.sync.dma_start(xq[:], xqT_d[:, :, qc * QW : (qc + 1) * QW])

            load_xq(0)
            nc.sync.dma_start(wk_sb[:], wkT_d[:])
            nc.sync.dma_start(wv_sb[:], wvT_d[:])
            load_slot(0)
            nc.sync.dma_start(mask_sb[:], mask_d[:])
            load_xq(1)
            load_xq(2)
            load_xq(3)
            nc.sync.dma_start(wo_sb[:], woT_d[:])
            load_slot(1)

            # inline: Q chunk 0 (slot 0 queries) and K^T head-pair 0 of slot 0
            for mt in range(NDT):
                q_chunk(0, mt)
            for c0 in range(0, tsched[0] * P, QW):
                k_chunk(0, 0, c0, min(c0 + QW, tsched[0] * P))

            # deferred: everything else, pumped into PE's ACT-bound gaps.
            # xq tiles rotate through 2 buffers: load qc+2 only after the
            # q_chunks of qc have certainly been emitted (WAR on the buffer).
            enq_kv_proj(0, (0, 0))
            for qc in range(1, 4):
                for mt in range(NDT):
                    enq((0, 0), (qc, 0), 853, q_chunk, (qc, mt))

            slot_ctxT = {}

            # ---- phases 1..NSLOT: attention per slot ----
            for s in range(NSLOT):
                t = tsched[s]
                tf = min(tfull[s], t)
                _, _, kT, v = slot_bufs[s]
                q0 = s * QW
                moff = slot_off[s]

                if s + 1 < NSLOT:
                    enq_kv_proj(s + 1, (s, 0))
                ctxT = ctp.tile([P, NDT, QW], BF16, tag="ctxT")
                slot_ctxT[s] = ctxT

                # exp groups: pairs over fully-valid tiles, singles (with
                # mask bias) over boundary/padding tiles
                npair = tf // 2
                groups = [(2 * i, 2, False) for i in range(npair)]
                groups += [(kt, 1, True) for kt in range(2 * npair, t)]

                # per-head engine-time estimates for budget pumping
                act_head = sum(1190 if n == 2 else 740 for _, n, _ in groups)
                pe_attn = t * 213 + 4 * t * 27 + 350

                cs_pairs = {}

                def pv_head(h, pt, t=t, v=v, ctxT=ctxT):
                    """PV + normalize + (pair) transpose for one head; one
                    PSUM accumulation group per bank."""
                    hm = h // 2
                    if h % 2 == 0:
                        cs_pairs[hm] = csp.tile([P, 4, 2, DH], BF16, tag="cs", name="cs")
                    cs_pair = cs_pairs[hm]
                    for j in range(4):
                        cxt = cxp.tile([P, QW], F32, tag="cx")
                        for kt in range(t):
                            nc.tensor.matmul(
                                cxt[:, 0 : DH + 1],
                                lhsT=pt[:, kt, j * P : (j + 1) * P],
                                rhs=v[:, kt, h, :],
                                start=(kt == 0),
                                stop=(kt == t - 1),
                            )
                        rc = rcp.tile([P, 1], F32, tag="rc")
                        nc.vector.reciprocal(rc[:], cxt[:, DH : DH + 1])
                        nc.vector.tensor_scalar_mul(
                            cs_pair[:, j, h % 2, :], cxt[:, 0:DH], rc[:]
                        )
                    if h % 2 == 1:
                        # transpose the head pair: [128q, 2*64] -> [128hd, 128q]
                        for j in range(4):
                            nc.sync.dma_start_transpose(
                                ctxT[:, hm, j * P : (j + 1) * P],
                                cs_pairs[hm][:, j, :, :].rearrange("p a b -> p (a b)"),
                            )

                gap_ns = max(0, act_head - pe_attn)
                prev = None
                for h in range(H):
                    pbase = (h % 2) * DH
                    hm = h // 2
                    pump_due((s, h))
                    pt = ptp.tile([P, 16, QW], BF16, tag="pt")
                    for gi, (g, n, masked) in enumerate(groups):
                        sc = scp.tile([P, 2, QW], F32, tag="sc")
                        for i in range(n):
                            nc.tensor.matmul(
                                sc[:, i, :],
                                lhsT=kT[pbase : pbase + DH, hm, (g + i) * P : (g + i + 1) * P],
                                rhs=qT_sb[pbase : pbase + DH, hm, q0 : q0 + QW],
                                start=True,
                                stop=True,
                            )
                        nc.scalar.activation(
                            pt[:, g : g + n, :].rearrange("p a b -> p (a b)"),
                            sc[:, :n, :].rearrange("p a b -> p (a b)"),
                            Exp,
                            bias=(mask_sb[:, moff + g : moff + g + 1] if masked else 0.0),
                            scale=0.125,
                        )
                        # fill the score-buffer-rotation stall with proj work
                        pump_budget((s, h), gap_ns * (gi + 1) // len(groups)
                                    - gap_ns * gi // len(groups))
                    if prev is not None:
                        pv_head(*prev)
                    prev = (h, pt)
                pv_head(*prev)

                for j in range(4):
                    enq((s + 1, 0), (s + 1, 2 * j + 1), 853, o_tile, (s, j))
                if s + 2 < NSLOT:
                    load_slot(s + 2)
            pump_due((NSLOT, H))

            for cm in reversed(_cms):
                cm.__exit__(None, None, None)

    nc.compile()
    return nc


_program_cache = {}


def _get_program(key=None):
    """test.py introspection helper: with no key, return the most recent."""
    if key is None:
        return next(reversed(_program_cache.values())) if _program_cache else None
    if key not in _program_cache:
        order, tsched, tfull = key[0], list(key[1]), list(key[2])
        _program_cache[key] = _build_program(tsched, tfull, key[3])
    return _program_cache[key]


def _tile_T(x):
    # [rows, 512] fp32 -> x^T [512, rows] -> [128, 4, rows] bf16
    xt = np.ascontiguousarray(x.T.astype(ml_dtypes.bfloat16))
    return np.ascontiguousarray(xt.reshape(NDT, P, x.shape[0]).transpose(1, 0, 2))


def kernel(**inputs):
    global last_results
    x_Q = np.asarray(inputs["x_Q"], dtype=np.float32)
    x_K = np.asarray(inputs["x_K"], dtype=np.float32)
    x_V = np.asarray(inputs["x_V"], dtype=np.float32)
    Wq = np.asarray(inputs["Wq"], dtype=np.float32)
    Wk = np.asarray(inputs["Wk"], dtype=np.float32)
    Wv = np.asarray(inputs["Wv"], dtype=np.float32)
    Wo = np.asarray(inputs["Wo"], dtype=np.float32)
    bq = np.asarray(inputs["bq"], dtype=np.float32)
    bk = np.asarray(inputs["bk"], dtype=np.float32)
    bv = np.asarray(inputs["bv"], dtype=np.float32)
    bo = np.asarray(inputs["bo"], dtype=np.float32)
    lens = np.asarray(inputs["src_batch_lens"]).astype(np.int64)

    zero_bias = bool(
        not bq.any() and not bk.any() and not bv.any() and not bo.any()
    )
    order, tsched, tfull = _schedule_from_lens(lens)
    key = (tuple(order), tuple(tsched), tuple(tfull), zero_bias)
    nc = _get_program(key)

    ktot = sum(tsched)
    slot_off = np.cumsum([0] + tsched)

    wqT = _tile_T(Wq)
    wkT = _tile_T(Wk)
    wvT = _tile_T(Wv)
    woT = _tile_T(Wo)
    if not zero_bias:
        bqT = np.ascontiguousarray(bq.reshape(NDT, P).T).astype(np.float32)
        bkT = np.ascontiguousarray(bk.reshape(NDT, P).T).astype(np.float32)
        bvb = np.ascontiguousarray(np.broadcast_to(bv, (P, D))).astype(np.float32)
        bob = np.ascontiguousarray(np.broadcast_to(bo, (P, D))).astype(np.float32)

    # zero out key/value rows at/beyond each batch's length (belt and
    # suspenders with the mask; required for the zero-bias fast path)
    kpos = np.arange(S)
    xk_z = np.where(kpos[None, :, None] < lens[:, None, None], x_K, 0.0)
    xv_z = np.where(kpos[None, :, None] < lens[:, None, None], x_V, 0.0)

    in_maps = []
    core_batches = []  # per core: list of batch ids per slot
    for c in range(B):
        batches = [int(order[2 * p + (0 if c < 4 else 1)]) for p in range(NSLOT)]
        qq = c % 4
        core_batches.append((batches, qq))

        xq_rows = np.concatenate(
            [x_Q[b, qq * QW : (qq + 1) * QW, :] for b in batches], axis=0
        )  # [2048, 512]
        xk_rows = np.zeros((ktot * P, D), np.float32)
        xv_rows = np.zeros((ktot * P, D), np.float32)
        mask = np.full((P, ktot), NEG, np.float32)
        for p, b in enumerate(batches):
            o0, o1 = slot_off[p] * P, slot_off[p + 1] * P
            nk = min(o1 - o0, S)
            xk_rows[o0 : o0 + nk] = xk_z[b, :nk]
            xv_rows[o0 : o0 + nk] = xv_z[b, :nk]
            kidx = (
                np.arange(slot_off[p] * P, slot_off[p + 1] * P)
                .reshape(-1, P)
                .T
                - o0
            )  # [128, T_p] key positions
            mask[:, slot_off[p] : slot_off[p + 1]] = np.where(
                kidx < lens[b], 0.0, NEG
            )

        im = {
            "xqT": _tile_T(xq_rows),
            "xkT": _tile_T(xk_rows),
            "xvT": _tile_T(xv_rows),
            "wqT": wqT,
            "wkT": wkT,
            "wvT": wvT,
            "woT": woT,
            "mask": np.ascontiguousarray(mask),
        }
        if not zero_bias:
            im.update({"bqT": bqT, "bkT": bkT, "bvb": bvb, "bob": bob})
        in_maps.append(im)

    res = run_bass_kernel_spmd(nc, in_maps, core_ids=list(range(B)))
    last_results = res

    out = np.empty((B, S, D), dtype=np.float32)
    for c in range(B):
        o = res.results[c]["out"]  # [128, 16, 512]
        batches, qq = core_batches[c]
        for p, b in enumerate(batches):
            for j in range(4):
                rows = slice(qq * QW + j * P, qq * QW + (j + 1) * P)
                out[b, rows, :] = o[:, p * 4 + j, :]
    return out
